# revision 2
# baseline (speedup 1.0000x reference)
"""Trainium2 Bass kernel for nn_Attention (dense transformer attention block).

Reference computation (shapes hardcoded):
  x [1, 256, 64, 64]; 1x1 conv+BN to q/k/v; 8 heads; per-head
  softmax(q @ k) @ v over n=4096 positions (dqk=32, dv=128); 1x1 conv+BN proj.

Sharding: one head per NeuronCore (8 cores). Each core computes its head's
attention plus its partial contribution to the projection conv
(z_h = Wp_h @ out_h); the host sums the 8 partials (the only cross-head
coupling is the channel-concat feeding the proj conv, which is a sum of
per-head matmuls).

Folding tricks (host-side, exact):
 - BN scales sq/sk fold into wq/wk rows; biases tq/tk applied on-device
   per-partition.
 - v's BN affine (sv, tv) folds through the attention (softmax rows sum to 1):
   sv scales wp columns, tv contributes a constant bias via wp @ tv.
 - proj BN affine (sp, tp) folds into wp rows / the constant bias.

On-device layouts avoid all transposes:
 - QT = [32, n] and K = [32, n] conv outputs replicated 4x along partitions
   so S^T tiles can be computed with 4-way row-packed (tile_position) matmuls.
 - S^T blocks [128 m, 512 q] -> exp (ScalarE) -> E fp16.
 - V computed directly transposed ([m, dv] blocks) via lhsT=X.
 - PV accumulates over m in PSUM; softmax denominator via col-packed
   ones-matmul (colsum) of E; normalization folded into the proj stage.

Matmul dtypes: float32r (full-rate fp32, ~1e-4 component error) for
QK/V/proj; fp16 for E/V in the PV and colsum passes (~3e-4).
"""

import numpy as np

import concourse.bacc as bacc
import concourse.mybir as mybir
import concourse.tile as tile
from concourse.bass_utils import run_bass_kernel_spmd

F32 = mybir.dt.float32
F32R = mybir.dt.float32r
FP16 = mybir.dt.float16
EXP = mybir.ActivationFunctionType.Exp

N = 4096          # positions (64*64)
C = 256           # input channels
H = 8             # heads
DQK = 32
DV = 128
NMB = N // 128    # 32 m-blocks
NQC = 8           # q-chunks of 512
QC = 512

_CACHED = {}


def _build_program():
    nc = bacc.Bacc(trn_type="TRN2")

    x2 = nc.dram_tensor("x2", [128, 2, N], F32R, kind="ExternalInput")
    wq_r = nc.dram_tensor("wq_r", [128, 2, 128], F32R, kind="ExternalInput")
    tq4 = nc.dram_tensor("tq4", [128, 1], F32, kind="ExternalInput")
    wk_r = nc.dram_tensor("wk_r", [128, 2, 128], F32R, kind="ExternalInput")
    tk4 = nc.dram_tensor("tk4", [128, 1], F32, kind="ExternalInput")
    wv_r = nc.dram_tensor("wv_r", [128, 2, 128], F32R, kind="ExternalInput")
    wp_r = nc.dram_tensor("wp_r", [128, 2, 128], F32R, kind="ExternalInput")
    bias2 = nc.dram_tensor("bias2", [128, 2], F32, kind="ExternalInput")
    z = nc.dram_tensor("z", [2, 128, N], F32, kind="ExternalOutput")

    with tile.TileContext(nc) as tc:
        with (
            tc.tile_pool(name="wgt", bufs=1) as wgt,
            tc.tile_pool(name="big", bufs=1) as big,
            tc.tile_pool(name="st_p", bufs=2, space="PSUM") as st_pool,
            tc.tile_pool(name="acc_p", bufs=1, space="PSUM") as acc_pool,
            tc.tile_pool(name="e_p", bufs=3) as e_pool,
            tc.tile_pool(name="misc", bufs=2) as misc,
        ):
            x2_s = wgt.tile([128, 2, N], F32R)
            nc.sync.dma_start(out=x2_s, in_=x2[:, :, :])
            wq_s = wgt.tile([128, 2, 128], F32R)
            nc.sync.dma_start(out=wq_s, in_=wq_r[:, :, :])
            wk_s = wgt.tile([128, 2, 128], F32R)
            nc.sync.dma_start(out=wk_s, in_=wk_r[:, :, :])
            wv_s = wgt.tile([128, 2, 128], F32R)
            nc.sync.dma_start(out=wv_s, in_=wv_r[:, :, :])
            wp_s = wgt.tile([128, 2, 128], F32R)
            nc.sync.dma_start(out=wp_s, in_=wp_r[:, :, :])
            tq_s = wgt.tile([128, 1], F32)
            nc.sync.dma_start(out=tq_s, in_=tq4[:, :])
            tk_s = wgt.tile([128, 1], F32)
            nc.sync.dma_start(out=tk_s, in_=tk4[:, :])
            bias_s = wgt.tile([128, 2], F32)
            nc.sync.dma_start(out=bias_s, in_=bias2[:, :])

            ones_f = wgt.tile([128, 32], F32)
            nc.vector.memset(ones_f, 1.0)
            ones = wgt.tile([128, 32], FP16)
            nc.vector.tensor_copy(ones, ones_f)

            # --- q/k 1x1 convs (+ bias), replicated 4x on partitions ---
            qt4 = big.tile([128, N], F32R)
            k4 = big.tile([128, N], F32R)
            for dst, w_s, t_s in ((qt4, wq_s, tq_s), (k4, wk_s, tk_s)):
                for n8 in range(NQC):
                    sl = slice(n8 * QC, (n8 + 1) * QC)
                    cp = st_pool.tile([128, QC], F32, tag="st")
                    nc.tensor.matmul(cp, w_s[:, 0, :], x2_s[:, 0, sl],
                                     start=True, stop=False)
                    nc.tensor.matmul(cp, w_s[:, 1, :], x2_s[:, 1, sl],
                                     start=False, stop=True)
                    nc.vector.tensor_scalar_add(dst[:, sl], cp, t_s[:, 0:1])

            # --- V, computed directly in [m, dv] layout (fp16) ---
            vt = big.tile([128, NMB, DV], FP16)
            for mb in range(NMB):
                msl = slice(mb * 128, (mb + 1) * 128)
                vp = st_pool.tile([128, DV], F32, tag="st")
                nc.tensor.matmul(vp, x2_s[:, 0, msl], wv_s[:, 0, :],
                                 start=True, stop=False)
                nc.tensor.matmul(vp, x2_s[:, 1, msl], wv_s[:, 1, :],
                                 start=False, stop=True)
                nc.vector.tensor_copy(vt[:, mb, :], vp)

            # --- main attention loop over q-chunks ---
            groups = [list(range(g, min(g + 3, NMB))) for g in range(0, NMB, 3)]
            for qc in range(NQC):
                qsl = slice(qc * QC, (qc + 1) * QC)
                out1 = acc_pool.tile([128, QC], F32, tag="out1")
                cs = acc_pool.tile([128, QC], F32, tag="cs")

                for group in groups:
                    st = st_pool.tile([128, 3, QC], F32, tag="st")
                    ng = len(group)
                    for i, mb in enumerate(group):
                        nc.tensor.matmul(
                            st[:, i, :],
                            k4[32 * i:32 * (i + 1), mb * 128:(mb + 1) * 128],
                            qt4[32 * i:32 * (i + 1), qsl],
                            start=True, stop=True,
                            tile_position=(32 * i, 0),
                        )
                    e = e_pool.tile([128, 3, QC], FP16, tag="e")
                    nc.scalar.activation(
                        out=e[:, 0:ng, :].rearrange("p a b -> p (a b)"),
                        in_=st[:, 0:ng, :].rearrange("p a b -> p (a b)"),
                        func=EXP,
                    )
                    for i, mb in enumerate(group):
                        nc.tensor.matmul(
                            out1, vt[:, mb, :], e[:, i, :],
                            start=(mb == 0), stop=(mb == NMB - 1),
                            skip_group_check=True,
                        )
                        j = mb % 4
                        nc.tensor.matmul(
                            cs[32 * j:32 * (j + 1), :],
                            ones[:, 0:32], e[:, i, :],
                            start=(mb < 4), stop=(mb >= NMB - 4),
                            tile_position=(0, 32 * j),
                            skip_group_check=True,
                        )

                # denominator: total = sum of 4 slabs (each 32 identical rows)
                cs_s = misc.tile([128, QC], F32, tag="cs_s")
                nc.vector.tensor_copy(cs_s, cs)
                slab = misc.tile([32, 3, QC], F32, tag="slab")
                for j in range(3):
                    nc.sync.dma_start(out=slab[:, j, :],
                                      in_=cs_s[32 * (j + 1):32 * (j + 2), :])
                t1 = misc.tile([32, QC], F32, tag="t1")
                nc.vector.tensor_add(t1, cs_s[0:32, :], slab[:, 0, :])
                t2 = misc.tile([32, QC], F32, tag="t2")
                nc.vector.tensor_add(t2, slab[:, 1, :], slab[:, 2, :])
                tot = misc.tile([32, QC], F32, tag="tot")
                nc.vector.tensor_add(tot, t1, t2)
                recipb = misc.tile([128, QC], F32, tag="recipb")
                nc.vector.reciprocal(recipb[0:32, :], tot)
                for j in range(1, 4):
                    nc.sync.dma_start(out=recipb[32 * j:32 * (j + 1), :],
                                      in_=recipb[0:32, :])

                out1n = misc.tile([128, QC], F32R, tag="out1n")
                nc.vector.tensor_mul(out1n, out1, recipb)

                for ob in range(2):
                    zp = st_pool.tile([128, QC], F32, tag="st")
                    nc.tensor.matmul(zp, wp_s[:, ob, :], out1n,
                                     start=True, stop=True)
                    zs = misc.tile([128, QC], F32, tag="zs")
                    nc.vector.tensor_scalar_add(zs, zp, bias_s[:, ob:ob + 1])
                    nc.sync.dma_start(out=z[ob, :, qsl], in_=zs)
    nc.compile()
    return nc


def _prepare_in_maps(x, wq, sq, tq, wk, sk, tk, wv, sv, tv, wp, sp, tp):
    X = np.ascontiguousarray(x.reshape(C, N), dtype=np.float32)
    x2 = np.ascontiguousarray(X.reshape(2, 128, N).transpose(1, 0, 2))
    bias_full = (sp * (wp @ tv) + tp) / H  # [256], per-core share

    in_maps = []
    for h in range(H):
        wq_h = (wq[32 * h:32 * (h + 1), :] * sq[32 * h:32 * (h + 1), None])
        wk_h = (wk[32 * h:32 * (h + 1), :] * sk[32 * h:32 * (h + 1), None])
        wqT_rep = np.tile(wq_h.T, (1, 4))        # [256, 128]
        wkT_rep = np.tile(wk_h.T, (1, 4))
        wq_r = np.ascontiguousarray(
            wqT_rep.reshape(2, 128, 128).transpose(1, 0, 2), dtype=np.float32)
        wk_r = np.ascontiguousarray(
            wkT_rep.reshape(2, 128, 128).transpose(1, 0, 2), dtype=np.float32)
        tq4 = np.ascontiguousarray(
            np.tile(tq[32 * h:32 * (h + 1)], 4).reshape(128, 1), dtype=np.float32)
        tk4 = np.ascontiguousarray(
            np.tile(tk[32 * h:32 * (h + 1)], 4).reshape(128, 1), dtype=np.float32)
        wv_h = wv[128 * h:128 * (h + 1), :]      # [dv, c]
        wv_r = np.ascontiguousarray(
            wv_h.T.reshape(2, 128, 128).transpose(1, 0, 2), dtype=np.float32)
        Wp_h = (sp[:, None] * wp[:, 128 * h:128 * (h + 1)]
                * sv[None, 128 * h:128 * (h + 1)])   # [256, 128]
        wp_r = np.ascontiguousarray(
            Wp_h.reshape(2, 128, 128).transpose(2, 0, 1), dtype=np.float32)
        bias2 = np.ascontiguousarray(
            bias_full.reshape(2, 128).T, dtype=np.float32)
        in_maps.append({
            "x2": x2, "wq_r": wq_r, "tq4": tq4, "wk_r": wk_r, "tk4": tk4,
            "wv_r": wv_r, "wp_r": wp_r, "bias2": bias2,
        })
    return in_maps


def kernel(**inputs):
    if "nc" not in _CACHED:
        _CACHED["nc"] = _build_program()
    nc = _CACHED["nc"]
    in_maps = _prepare_in_maps(**inputs)
    res = run_bass_kernel_spmd(nc, in_maps, core_ids=list(range(H)))
    y = np.zeros((2, 128, N), dtype=np.float64)
    for r in res.results:
        y += r["z"]
    return y.reshape(C, N).reshape(1, C, 64, 64).astype(np.float32)


# revision 3
# speedup vs baseline: 1.1362x; 1.1362x over previous
"""Trainium2 Bass kernel for nn_Attention (dense transformer attention block).

Reference computation (shapes hardcoded):
  x [1, 256, 64, 64]; 1x1 conv+BN to q/k/v; 8 heads; per-head
  softmax(q @ k) @ v over n=4096 positions (dqk=32, dv=128); 1x1 conv+BN proj.

Sharding: one head per NeuronCore (8 cores). Each core computes its head's
attention plus its partial contribution to the projection conv
(z_h = Wp_h @ out_h); the host sums the 8 partials (the only cross-head
coupling is the channel-concat feeding the proj conv, which is a sum of
per-head matmuls).

Folding tricks (host-side, exact):
 - BN scales sq/sk fold into wq/wk rows; biases tq/tk applied on-device
   per-partition.
 - v's BN affine (sv, tv) folds through the attention (softmax rows sum to 1):
   sv scales wp columns, tv contributes a constant bias via wp @ tv.
 - proj BN affine (sp, tp) folds into wp rows / the constant bias.

On-device layouts avoid all transposes:
 - QT = [32, n] and K = [32, n] conv outputs replicated 4x along partitions
   so S^T tiles can be computed with 4-way row-packed (tile_position) matmuls.
 - S^T blocks [128 m, 512 q] -> exp (ScalarE) -> E fp16.
 - V computed directly transposed ([m, dv] blocks) via lhsT=X.
 - PV accumulates over m in PSUM; softmax denominator via col-packed
   ones-matmul (colsum) of E; normalization folded into the proj stage.

Matmul dtypes: float32r (full-rate fp32, ~1e-4 component error) for
QK/V/proj; fp16 for E/V in the PV and colsum passes (~3e-4).
"""

import numpy as np

import concourse.bacc as bacc
import concourse.mybir as mybir
import concourse.tile as tile
from concourse.bass_utils import run_bass_kernel_spmd

F32 = mybir.dt.float32
F32R = mybir.dt.float32r
FP16 = mybir.dt.float16
EXP = mybir.ActivationFunctionType.Exp

N = 4096          # positions (64*64)
C = 256           # input channels
H = 8             # heads
DQK = 32
DV = 128
NMB = N // 128    # 32 m-blocks
NQC = 8           # q-chunks of 512
QC = 512

_CACHED = {}


def _build_program():
    nc = bacc.Bacc(trn_type="TRN2")

    x2 = nc.dram_tensor("x2", [128, 2, N], F32R, kind="ExternalInput")
    wq_r = nc.dram_tensor("wq_r", [128, 2, 128], F32R, kind="ExternalInput")
    tq4 = nc.dram_tensor("tq4", [128, 1], F32, kind="ExternalInput")
    wk_r = nc.dram_tensor("wk_r", [128, 2, 128], F32R, kind="ExternalInput")
    tk4 = nc.dram_tensor("tk4", [128, 1], F32, kind="ExternalInput")
    wv_r = nc.dram_tensor("wv_r", [128, 2, 128], F32R, kind="ExternalInput")
    wp_r = nc.dram_tensor("wp_r", [128, 2, 128], F32R, kind="ExternalInput")
    bias2 = nc.dram_tensor("bias2", [128, 2], F32, kind="ExternalInput")
    z = nc.dram_tensor("z", [2, 128, N], F32, kind="ExternalOutput")

    with tile.TileContext(nc) as tc:
        with (
            tc.tile_pool(name="wgt", bufs=1) as wgt,
            tc.tile_pool(name="big", bufs=1) as big,
            tc.tile_pool(name="st_p", bufs=1, space="PSUM") as st_pool,
            tc.tile_pool(name="acc_p", bufs=2, space="PSUM") as acc_pool,
            tc.tile_pool(name="zp_p", bufs=1, space="PSUM") as zp_pool,
            tc.tile_pool(name="e_p", bufs=3) as e_pool,
            tc.tile_pool(name="misc", bufs=2) as misc,
        ):
            x2_s = wgt.tile([128, 2, N], F32R)
            nc.sync.dma_start(out=x2_s, in_=x2[:, :, :])
            wq_s = wgt.tile([128, 2, 128], F32R)
            nc.sync.dma_start(out=wq_s, in_=wq_r[:, :, :])
            wk_s = wgt.tile([128, 2, 128], F32R)
            nc.sync.dma_start(out=wk_s, in_=wk_r[:, :, :])
            wv_s = wgt.tile([128, 2, 128], F32R)
            nc.sync.dma_start(out=wv_s, in_=wv_r[:, :, :])
            wp_s = wgt.tile([128, 2, 128], F32R)
            nc.sync.dma_start(out=wp_s, in_=wp_r[:, :, :])
            tq_s = wgt.tile([128, 1], F32)
            nc.sync.dma_start(out=tq_s, in_=tq4[:, :])
            tk_s = wgt.tile([128, 1], F32)
            nc.sync.dma_start(out=tk_s, in_=tk4[:, :])
            bias_s = wgt.tile([128, 2], F32)
            nc.sync.dma_start(out=bias_s, in_=bias2[:, :])

            ones_f = wgt.tile([128, 32], F32)
            nc.vector.memset(ones_f, 1.0)
            ones = wgt.tile([128, 32], FP16)
            nc.vector.tensor_copy(ones, ones_f)

            # --- q/k 1x1 convs (+ bias), replicated 4x on partitions ---
            qt4 = big.tile([128, N], F32R)
            k4 = big.tile([128, N], F32R)
            for dst, w_s, t_s in ((qt4, wq_s, tq_s), (k4, wk_s, tk_s)):
                for n8 in range(NQC):
                    sl = slice(n8 * QC, (n8 + 1) * QC)
                    cp = st_pool.tile([128, QC], F32, tag="st")
                    nc.tensor.matmul(cp, w_s[:, 0, :], x2_s[:, 0, sl],
                                     start=True, stop=False)
                    nc.tensor.matmul(cp, w_s[:, 1, :], x2_s[:, 1, sl],
                                     start=False, stop=True)
                    nc.vector.tensor_scalar_add(dst[:, sl], cp, t_s[:, 0:1])

            # --- V, computed directly in [m, dv] layout (fp16) ---
            vt = big.tile([128, NMB, DV], FP16)
            for mb in range(NMB):
                msl = slice(mb * 128, (mb + 1) * 128)
                vp = st_pool.tile([128, DV], F32, tag="st")
                nc.tensor.matmul(vp, x2_s[:, 0, msl], wv_s[:, 0, :],
                                 start=True, stop=False)
                nc.tensor.matmul(vp, x2_s[:, 1, msl], wv_s[:, 1, :],
                                 start=False, stop=True)
                nc.vector.tensor_copy(vt[:, mb, :], vp)

            # --- main attention loop over q-chunks ---
            groups = [list(range(g, min(g + 3, NMB))) for g in range(0, NMB, 3)]
            for qc in range(NQC):
                qsl = slice(qc * QC, (qc + 1) * QC)
                out1 = acc_pool.tile([128, QC], F32, tag="out1")
                cs = acc_pool.tile([128, QC], F32, tag="cs")

                for group in groups:
                    st = st_pool.tile([128, 3, QC], F32, tag="st")
                    ng = len(group)
                    for i, mb in enumerate(group):
                        nc.tensor.matmul(
                            st[:, i, :],
                            k4[32 * i:32 * (i + 1), mb * 128:(mb + 1) * 128],
                            qt4[32 * i:32 * (i + 1), qsl],
                            start=True, stop=True,
                            tile_position=(32 * i, 0),
                        )
                    e = e_pool.tile([128, 3, QC], FP16, tag="e")
                    nc.scalar.activation(
                        out=e[:, 0:ng, :].rearrange("p a b -> p (a b)"),
                        in_=st[:, 0:ng, :].rearrange("p a b -> p (a b)"),
                        func=EXP,
                    )
                    for i, mb in enumerate(group):
                        nc.tensor.matmul(
                            out1, vt[:, mb, :], e[:, i, :],
                            start=(mb == 0), stop=(mb == NMB - 1),
                            skip_group_check=True,
                        )
                    for i, mb in enumerate(group):
                        j = mb % 4
                        nc.tensor.matmul(
                            cs[32 * j:32 * (j + 1), :],
                            ones[:, 0:32], e[:, i, :],
                            start=(mb < 4), stop=(mb >= NMB - 4),
                            tile_position=(0, 32 * j),
                            skip_group_check=True,
                        )

                # denominator: total = sum of 4 slabs (each 32 identical rows)
                cs_s = misc.tile([128, QC], F32, tag="cs_s")
                nc.vector.tensor_copy(cs_s, cs)
                slab = misc.tile([32, 3, QC], F32, tag="slab")
                for j in range(3):
                    nc.sync.dma_start(out=slab[:, j, :],
                                      in_=cs_s[32 * (j + 1):32 * (j + 2), :])
                t1 = misc.tile([32, QC], F32, tag="t1")
                nc.vector.tensor_add(t1, cs_s[0:32, :], slab[:, 0, :])
                t2 = misc.tile([32, QC], F32, tag="t2")
                nc.vector.tensor_add(t2, slab[:, 1, :], slab[:, 2, :])
                tot = misc.tile([32, QC], F32, tag="tot")
                nc.vector.tensor_add(tot, t1, t2)
                recipb = misc.tile([128, QC], F32, tag="recipb")
                nc.vector.reciprocal(recipb[0:32, :], tot)
                for j in range(1, 4):
                    nc.sync.dma_start(out=recipb[32 * j:32 * (j + 1), :],
                                      in_=recipb[0:32, :])

                out1n = misc.tile([128, QC], F32R, tag="out1n")
                nc.vector.tensor_mul(out1n, out1, recipb)

                for ob in range(2):
                    zp = zp_pool.tile([128, QC], F32, tag="zp")
                    nc.tensor.matmul(zp, wp_s[:, ob, :], out1n,
                                     start=True, stop=True)
                    zs = misc.tile([128, QC], F32, tag="zs")
                    nc.vector.tensor_scalar_add(zs, zp, bias_s[:, ob:ob + 1])
                    nc.sync.dma_start(out=z[ob, :, qsl], in_=zs)
    nc.compile()
    return nc


def _prepare_in_maps(x, wq, sq, tq, wk, sk, tk, wv, sv, tv, wp, sp, tp):
    X = np.ascontiguousarray(x.reshape(C, N), dtype=np.float32)
    x2 = np.ascontiguousarray(X.reshape(2, 128, N).transpose(1, 0, 2))
    bias_full = (sp * (wp @ tv) + tp) / H  # [256], per-core share

    in_maps = []
    for h in range(H):
        wq_h = (wq[32 * h:32 * (h + 1), :] * sq[32 * h:32 * (h + 1), None])
        wk_h = (wk[32 * h:32 * (h + 1), :] * sk[32 * h:32 * (h + 1), None])
        wqT_rep = np.tile(wq_h.T, (1, 4))        # [256, 128]
        wkT_rep = np.tile(wk_h.T, (1, 4))
        wq_r = np.ascontiguousarray(
            wqT_rep.reshape(2, 128, 128).transpose(1, 0, 2), dtype=np.float32)
        wk_r = np.ascontiguousarray(
            wkT_rep.reshape(2, 128, 128).transpose(1, 0, 2), dtype=np.float32)
        tq4 = np.ascontiguousarray(
            np.tile(tq[32 * h:32 * (h + 1)], 4).reshape(128, 1), dtype=np.float32)
        tk4 = np.ascontiguousarray(
            np.tile(tk[32 * h:32 * (h + 1)], 4).reshape(128, 1), dtype=np.float32)
        wv_h = wv[128 * h:128 * (h + 1), :]      # [dv, c]
        wv_r = np.ascontiguousarray(
            wv_h.T.reshape(2, 128, 128).transpose(1, 0, 2), dtype=np.float32)
        Wp_h = (sp[:, None] * wp[:, 128 * h:128 * (h + 1)]
                * sv[None, 128 * h:128 * (h + 1)])   # [256, 128]
        wp_r = np.ascontiguousarray(
            Wp_h.reshape(2, 128, 128).transpose(2, 0, 1), dtype=np.float32)
        bias2 = np.ascontiguousarray(
            bias_full.reshape(2, 128).T, dtype=np.float32)
        in_maps.append({
            "x2": x2, "wq_r": wq_r, "tq4": tq4, "wk_r": wk_r, "tk4": tk4,
            "wv_r": wv_r, "wp_r": wp_r, "bias2": bias2,
        })
    return in_maps


def kernel(**inputs):
    if "nc" not in _CACHED:
        _CACHED["nc"] = _build_program()
    nc = _CACHED["nc"]
    in_maps = _prepare_in_maps(**inputs)
    res = run_bass_kernel_spmd(nc, in_maps, core_ids=list(range(H)))
    y = np.zeros((2, 128, N), dtype=np.float64)
    for r in res.results:
        y += r["z"]
    return y.reshape(C, N).reshape(1, C, 64, 64).astype(np.float32)


# revision 5
# speedup vs baseline: 2.1773x; 1.9163x over previous
"""Trainium2 Bass kernel for nn_Attention (dense transformer attention block).

Reference computation (shapes hardcoded):
  x [1, 256, 64, 64]; 1x1 conv+BN to q/k/v; 8 heads; per-head
  softmax(q @ k) @ v over n=4096 positions (dqk=32, dv=128); 1x1 conv+BN proj.

Sharding: one head per NeuronCore (8 cores). Each core computes its head's
attention plus its partial contribution to the projection conv
(z_h = Wp_h @ out_h); the host sums the 8 partials (the only cross-head
coupling is the channel-concat feeding the proj conv, which is a sum of
per-head matmuls).

Folding tricks (host-side, exact):
 - BN scales sq/sk fold into wq/wk rows; biases tq/tk applied on-device
   per-partition.
 - v's BN affine (sv, tv) folds through the attention (softmax rows sum to 1):
   sv scales wp columns, tv contributes a constant bias via wp @ tv.
 - proj BN affine (sp, tp) folds into wp rows / the constant bias.

On-device layouts avoid all transposes:
 - QT = [32, n] and K = [32, n] conv outputs replicated 4x along partitions
   so S^T tiles can be computed with 4-way row-packed (tile_position) matmuls.
 - S^T blocks [128 m, 512 q] -> exp (ScalarE) -> E fp16.
 - V computed directly transposed ([m, dv] blocks) via lhsT=X.
 - PV accumulates over m in PSUM; softmax denominator via col-packed
   ones-matmul (colsum) of E; normalization folded into the proj stage.

Matmul dtypes: float32r (full-rate fp32, ~1e-4 component error) for
QK/V/proj; fp16 for E/V in the PV and colsum passes (~3e-4).
"""

import numpy as np

import concourse.bacc as bacc
import concourse.mybir as mybir
import concourse.tile as tile
from concourse.bass_utils import run_bass_kernel_spmd

F32 = mybir.dt.float32
F32R = mybir.dt.float32r
FP16 = mybir.dt.float16
EXP = mybir.ActivationFunctionType.Exp

N = 4096          # positions (64*64)
C = 256           # input channels
H = 8             # heads
DQK = 32
DV = 128
NMB = N // 128    # 32 m-blocks
NQC = 8           # q-chunks of 512
QC = 512

_CACHED = {}


def _build_program():
    nc = bacc.Bacc(trn_type="TRN2")

    x2 = nc.dram_tensor("x2", [128, 2, N], F32R, kind="ExternalInput")
    wq_r = nc.dram_tensor("wq_r", [128, 2, 128], F32R, kind="ExternalInput")
    tq4 = nc.dram_tensor("tq4", [128, 1], F32, kind="ExternalInput")
    wk_r = nc.dram_tensor("wk_r", [128, 2, 128], F32R, kind="ExternalInput")
    tk4 = nc.dram_tensor("tk4", [128, 1], F32, kind="ExternalInput")
    wv_r = nc.dram_tensor("wv_r", [128, 2, 128], F32R, kind="ExternalInput")
    wp_r = nc.dram_tensor("wp_r", [128, 2, 128], F32R, kind="ExternalInput")
    bias2 = nc.dram_tensor("bias2", [128, 2], F32, kind="ExternalInput")
    z = nc.dram_tensor("z", [2, 128, N], F32, kind="ExternalOutput")

    with tile.TileContext(nc) as tc:
        with (
            tc.tile_pool(name="wgt", bufs=1) as wgt,
            tc.tile_pool(name="big", bufs=1) as big,
            tc.tile_pool(name="st_p", bufs=2, space="PSUM") as st_pool,
            tc.tile_pool(name="acc_p", bufs=1, space="PSUM") as acc_pool,
            tc.tile_pool(name="e_p", bufs=3) as e_pool,
            tc.tile_pool(name="misc", bufs=2) as misc,
        ):
            x2_s = wgt.tile([128, 2, N], F32R)
            for nq in range(4):
                nc.sync.dma_start(
                    out=x2_s[:, :, nq * 1024:(nq + 1) * 1024],
                    in_=x2[:, :, nq * 1024:(nq + 1) * 1024])
            wq_s = wgt.tile([128, 2, 128], F32R)
            nc.sync.dma_start(out=wq_s, in_=wq_r[:, :, :])
            wk_s = wgt.tile([128, 2, 128], F32R)
            nc.sync.dma_start(out=wk_s, in_=wk_r[:, :, :])
            wv_s = wgt.tile([128, 2, 128], F32R)
            nc.sync.dma_start(out=wv_s, in_=wv_r[:, :, :])
            wp_s = wgt.tile([128, 2, 128], F32R)
            nc.sync.dma_start(out=wp_s, in_=wp_r[:, :, :])
            tq_s = wgt.tile([128, 1], F32)
            nc.sync.dma_start(out=tq_s, in_=tq4[:, :])
            tk_s = wgt.tile([128, 1], F32)
            nc.sync.dma_start(out=tk_s, in_=tk4[:, :])
            bias_s = wgt.tile([128, 2], F32)
            nc.sync.dma_start(out=bias_s, in_=bias2[:, :])

            ones_f = wgt.tile([128, 32], F32)
            nc.vector.memset(ones_f, 1.0)
            ones = wgt.tile([128, 32], FP16)
            nc.vector.tensor_copy(ones, ones_f)

            # --- q/k 1x1 convs (+ bias), replicated 4x on partitions ---
            qt4 = big.tile([128, N], F32R)
            k4 = big.tile([128, N], F32R)
            for dst, w_s, t_s in ((qt4, wq_s, tq_s), (k4, wk_s, tk_s)):
                for n8 in range(NQC):
                    sl = slice(n8 * QC, (n8 + 1) * QC)
                    cp = st_pool.tile([128, QC], F32, tag="st")
                    nc.tensor.matmul(cp, w_s[:, 0, :], x2_s[:, 0, sl],
                                     start=True, stop=False)
                    nc.tensor.matmul(cp, w_s[:, 1, :], x2_s[:, 1, sl],
                                     start=False, stop=True)
                    nc.vector.tensor_scalar_add(dst[:, sl], cp, t_s[:, 0:1])

            # --- V tiles [m, dv] fp16, computed inside chunk 0's m-loop ---
            vt = big.tile([128, NMB, DV], FP16)

            def emit_vt(mb):
                msl = slice(mb * 128, (mb + 1) * 128)
                vp = st_pool.tile([128, DV], F32, tag="st")
                nc.tensor.matmul(vp, x2_s[:, 0, msl], wv_s[:, 0, :],
                                 start=True, stop=False)
                nc.tensor.matmul(vp, x2_s[:, 1, msl], wv_s[:, 1, :],
                                 start=False, stop=True)
                nc.vector.tensor_copy(vt[:, mb, :], vp)

            # --- main attention loop over q-chunks (software-pipelined) ---
            groups = [list(range(g, min(g + 3, NMB))) for g in range(0, NMB, 3)]
            out1n_all = big.tile([128, NQC, QC], F32R)

            def emit_st(qsl, group):
                st = st_pool.tile([128, 3, QC], F32, tag="st")
                for i, mb in enumerate(group):
                    nc.tensor.matmul(
                        st[:, i, :],
                        k4[32 * i:32 * (i + 1), mb * 128:(mb + 1) * 128],
                        qt4[32 * i:32 * (i + 1), qsl],
                        start=True, stop=True,
                        tile_position=(32 * i, 0),
                    )
                return st

            for qc in range(NQC):
                qsl = slice(qc * QC, (qc + 1) * QC)
                out1 = acc_pool.tile([128, QC], F32, tag="out1")
                cs = acc_pool.tile([128, QC], F32, tag="cs")

                st_cur = emit_st(qsl, groups[0])
                for g, group in enumerate(groups):
                    ng = len(group)
                    e = e_pool.tile([128, 3, QC], FP16, tag="e")
                    nc.scalar.activation(
                        out=e[:, 0:ng, :].rearrange("p a b -> p (a b)"),
                        in_=st_cur[:, 0:ng, :].rearrange("p a b -> p (a b)"),
                        func=EXP,
                    )
                    if qc == 0:
                        for mb in group:
                            emit_vt(mb)
                    if g + 1 < len(groups):
                        st_cur = emit_st(qsl, groups[g + 1])
                    for i, mb in enumerate(group):
                        nc.tensor.matmul(
                            out1, vt[:, mb, :], e[:, i, :],
                            start=(mb == 0), stop=(mb == NMB - 1),
                            skip_group_check=True,
                        )
                    for i, mb in enumerate(group):
                        j = mb % 4
                        nc.tensor.matmul(
                            cs[32 * j:32 * (j + 1), :],
                            ones[:, 0:32], e[:, i, :],
                            start=(mb < 4), stop=(mb >= NMB - 4),
                            tile_position=(0, 32 * j),
                            skip_group_check=True,
                        )

                # epilogue (SBUF-side; psum banks released by the two copies)
                cs_s = misc.tile([128, QC], F32, tag="cs_s")
                nc.vector.tensor_copy(cs_s, cs)
                out1_s = misc.tile([128, QC], F32, tag="out1_s")
                nc.vector.tensor_copy(out1_s, out1)
                slab = misc.tile([32, 3, QC], F32, tag="slab")
                for j in range(3):
                    nc.sync.dma_start(out=slab[:, j, :],
                                      in_=cs_s[32 * (j + 1):32 * (j + 2), :])
                t1 = misc.tile([32, QC], F32, tag="t1")
                nc.vector.tensor_add(t1, cs_s[0:32, :], slab[:, 0, :])
                t2 = misc.tile([32, QC], F32, tag="t2")
                nc.vector.tensor_add(t2, slab[:, 1, :], slab[:, 2, :])
                tot = misc.tile([32, QC], F32, tag="tot")
                nc.vector.tensor_add(tot, t1, t2)
                recipb = misc.tile([128, QC], F32, tag="recipb")
                nc.vector.reciprocal(recipb[0:32, :], tot)
                for j in range(1, 4):
                    nc.sync.dma_start(out=recipb[32 * j:32 * (j + 1), :],
                                      in_=recipb[0:32, :])
                nc.vector.tensor_mul(out1n_all[:, qc, :], out1_s, recipb)

            # --- projection tail: z[ob] = Wp_h[ob].T @ out1n (+bias) ---
            qtrios = [[0, 1, 2], [3, 4, 5], [6, 7]]
            for ob in range(2):
                for qtrio in qtrios:
                    nq = len(qtrio)
                    zp = st_pool.tile([128, 3, QC], F32, tag="st")
                    for j, qc in enumerate(qtrio):
                        nc.tensor.matmul(zp[:, j, :], wp_s[:, ob, :],
                                         out1n_all[:, qc, :],
                                         start=True, stop=True)
                    zs = misc.tile([128, 3, QC], F32, tag="zs")
                    nc.vector.tensor_scalar_add(
                        zs[:, 0:nq, :].rearrange("p a b -> p (a b)"),
                        zp[:, 0:nq, :].rearrange("p a b -> p (a b)"),
                        bias_s[:, ob:ob + 1])
                    nc.sync.dma_start(
                        out=z[ob, :, qtrio[0] * QC:(qtrio[-1] + 1) * QC],
                        in_=zs[:, 0:nq, :].rearrange("p a b -> p (a b)"))
    nc.compile()
    return nc


def _prepare_in_maps(x, wq, sq, tq, wk, sk, tk, wv, sv, tv, wp, sp, tp):
    X = np.ascontiguousarray(x.reshape(C, N), dtype=np.float32)
    x2 = np.ascontiguousarray(X.reshape(2, 128, N).transpose(1, 0, 2))
    bias_full = (sp * (wp @ tv) + tp) / H  # [256], per-core share

    in_maps = []
    for h in range(H):
        wq_h = (wq[32 * h:32 * (h + 1), :] * sq[32 * h:32 * (h + 1), None])
        wk_h = (wk[32 * h:32 * (h + 1), :] * sk[32 * h:32 * (h + 1), None])
        wqT_rep = np.tile(wq_h.T, (1, 4))        # [256, 128]
        wkT_rep = np.tile(wk_h.T, (1, 4))
        wq_r = np.ascontiguousarray(
            wqT_rep.reshape(2, 128, 128).transpose(1, 0, 2), dtype=np.float32)
        wk_r = np.ascontiguousarray(
            wkT_rep.reshape(2, 128, 128).transpose(1, 0, 2), dtype=np.float32)
        tq4 = np.ascontiguousarray(
            np.tile(tq[32 * h:32 * (h + 1)], 4).reshape(128, 1), dtype=np.float32)
        tk4 = np.ascontiguousarray(
            np.tile(tk[32 * h:32 * (h + 1)], 4).reshape(128, 1), dtype=np.float32)
        wv_h = wv[128 * h:128 * (h + 1), :]      # [dv, c]
        wv_r = np.ascontiguousarray(
            wv_h.T.reshape(2, 128, 128).transpose(1, 0, 2), dtype=np.float32)
        Wp_h = (sp[:, None] * wp[:, 128 * h:128 * (h + 1)]
                * sv[None, 128 * h:128 * (h + 1)])   # [256, 128]
        wp_r = np.ascontiguousarray(
            Wp_h.reshape(2, 128, 128).transpose(2, 0, 1), dtype=np.float32)
        bias2 = np.ascontiguousarray(
            bias_full.reshape(2, 128).T, dtype=np.float32)
        in_maps.append({
            "x2": x2, "wq_r": wq_r, "tq4": tq4, "wk_r": wk_r, "tk4": tk4,
            "wv_r": wv_r, "wp_r": wp_r, "bias2": bias2,
        })
    return in_maps


def kernel(**inputs):
    if "nc" not in _CACHED:
        _CACHED["nc"] = _build_program()
    nc = _CACHED["nc"]
    in_maps = _prepare_in_maps(**inputs)
    res = run_bass_kernel_spmd(nc, in_maps, core_ids=list(range(H)))
    y = np.zeros((2, 128, N), dtype=np.float64)
    for r in res.results:
        y += r["z"]
    return y.reshape(C, N).reshape(1, C, 64, 64).astype(np.float32)


# revision 6
# speedup vs baseline: 2.2607x; 1.0383x over previous
"""Trainium2 Bass kernel for nn_Attention (dense transformer attention block).

Reference computation (shapes hardcoded):
  x [1, 256, 64, 64]; 1x1 conv+BN to q/k/v; 8 heads; per-head
  softmax(q @ k) @ v over n=4096 positions (dqk=32, dv=128); 1x1 conv+BN proj.

Sharding: one head per NeuronCore (8 cores). Each core computes its head's
attention plus its partial contribution to the projection conv
(z_h = Wp_h @ out_h); the host sums the 8 partials (the only cross-head
coupling is the channel-concat feeding the proj conv, which is a sum of
per-head matmuls).

Folding tricks (host-side, exact):
 - BN scales sq/sk fold into wq/wk rows; biases tq/tk applied on-device
   per-partition.
 - v's BN affine (sv, tv) folds through the attention (softmax rows sum to 1):
   sv scales wp columns, tv contributes a constant bias via wp @ tv.
 - proj BN affine (sp, tp) folds into wp rows / the constant bias.

On-device layouts avoid all transposes:
 - QT = [32, n] and K = [32, n] conv outputs replicated 4x along partitions
   so S^T tiles can be computed with 4-way row-packed (tile_position) matmuls.
 - S^T blocks [128 m, 512 q] -> exp (ScalarE) -> E fp16.
 - V computed directly transposed ([m, dv] blocks) via lhsT=X.
 - PV accumulates over m in PSUM; softmax denominator via col-packed
   ones-matmul (colsum) of E; normalization folded into the proj stage.

Matmul dtypes: float32r (full-rate fp32, ~1e-4 component error) for
QK/V/proj; fp16 for E/V in the PV and colsum passes (~3e-4).
"""

import numpy as np

import concourse.bacc as bacc
import concourse.mybir as mybir
import concourse.tile as tile
from concourse.bass_utils import run_bass_kernel_spmd

F32 = mybir.dt.float32
F32R = mybir.dt.float32r
FP16 = mybir.dt.float16
EXP = mybir.ActivationFunctionType.Exp

N = 4096          # positions (64*64)
C = 256           # input channels
H = 8             # heads
DQK = 32
DV = 128
NMB = N // 128    # 32 m-blocks
NQC = 8           # q-chunks of 512
QC = 512

_CACHED = {}


def _build_program():
    nc = bacc.Bacc(trn_type="TRN2")

    x2 = nc.dram_tensor("x2", [128, 2, N], F32R, kind="ExternalInput")
    wq_r = nc.dram_tensor("wq_r", [128, 2, 128], F32R, kind="ExternalInput")
    tq4 = nc.dram_tensor("tq4", [128, 1], F32, kind="ExternalInput")
    wk_r = nc.dram_tensor("wk_r", [128, 2, 128], F32R, kind="ExternalInput")
    tk4 = nc.dram_tensor("tk4", [128, 1], F32, kind="ExternalInput")
    wv_r = nc.dram_tensor("wv_r", [128, 2, 128], F32R, kind="ExternalInput")
    wp_r = nc.dram_tensor("wp_r", [128, 2, 128], F32R, kind="ExternalInput")
    bias2 = nc.dram_tensor("bias2", [128, 2], F32, kind="ExternalInput")
    z = nc.dram_tensor("z", [2, 128, N], F32, kind="ExternalOutput")

    with tile.TileContext(nc) as tc:
        with (
            tc.tile_pool(name="wgt", bufs=1) as wgt,
            tc.tile_pool(name="big", bufs=1) as big,
            tc.tile_pool(name="st_p", bufs=2, space="PSUM") as st_pool,
            tc.tile_pool(name="acc_p", bufs=1, space="PSUM") as acc_pool,
            tc.tile_pool(name="e_p", bufs=3) as e_pool,
            tc.tile_pool(name="misc", bufs=2) as misc,
        ):
            wq_s = wgt.tile([128, 2, 128], F32R)
            nc.sync.dma_start(out=wq_s, in_=wq_r[:, :, :])
            wk_s = wgt.tile([128, 2, 128], F32R)
            nc.sync.dma_start(out=wk_s, in_=wk_r[:, :, :])
            wv_s = wgt.tile([128, 2, 128], F32R)
            nc.sync.dma_start(out=wv_s, in_=wv_r[:, :, :])
            wp_s = wgt.tile([128, 2, 128], F32R)
            nc.sync.dma_start(out=wp_s, in_=wp_r[:, :, :])
            tq_s = wgt.tile([128, 1], F32)
            nc.sync.dma_start(out=tq_s, in_=tq4[:, :])
            tk_s = wgt.tile([128, 1], F32)
            nc.sync.dma_start(out=tk_s, in_=tk4[:, :])
            bias_s = wgt.tile([128, 2], F32)
            nc.sync.dma_start(out=bias_s, in_=bias2[:, :])
            x2_s = wgt.tile([128, 2, N], F32R)
            for nq in range(4):
                nc.sync.dma_start(
                    out=x2_s[:, :, nq * 1024:(nq + 1) * 1024],
                    in_=x2[:, :, nq * 1024:(nq + 1) * 1024])

            ones_f = wgt.tile([128, 32], F32)
            nc.vector.memset(ones_f, 1.0)
            ones = wgt.tile([128, 32], FP16)
            nc.vector.tensor_copy(ones, ones_f)

            # --- q/k 1x1 convs (+ bias), replicated 4x on partitions.
            # Only the chunks needed for q-chunk 0 / trio 0 are emitted up
            # front; the rest stream into chunk 0's m-loop.
            qt4 = big.tile([128, N], F32R)
            k4 = big.tile([128, N], F32R)

            def emit_conv_chunk(dst, w_s, t_s, n8):
                sl = slice(n8 * QC, (n8 + 1) * QC)
                cp = st_pool.tile([128, QC], F32, tag="st")
                nc.tensor.matmul(cp, w_s[:, 0, :], x2_s[:, 0, sl],
                                 start=True, stop=False)
                nc.tensor.matmul(cp, w_s[:, 1, :], x2_s[:, 1, sl],
                                 start=False, stop=True)
                nc.vector.tensor_scalar_add(dst[:, sl], cp, t_s[:, 0:1])

            emit_conv_chunk(qt4, wq_s, tq_s, 0)
            emit_conv_chunk(k4, wk_s, tk_s, 0)
            k4_chunks_done = 1
            qt4_chunks_done = 1

            # --- V tiles [m, dv] fp16, computed inside chunk 0's m-loop ---
            vt = big.tile([128, NMB, DV], FP16)

            def emit_vt(mb):
                msl = slice(mb * 128, (mb + 1) * 128)
                vp = st_pool.tile([128, DV], F32, tag="st")
                nc.tensor.matmul(vp, x2_s[:, 0, msl], wv_s[:, 0, :],
                                 start=True, stop=False)
                nc.tensor.matmul(vp, x2_s[:, 1, msl], wv_s[:, 1, :],
                                 start=False, stop=True)
                nc.vector.tensor_copy(vt[:, mb, :], vp)

            # --- main attention loop over q-chunks (software-pipelined) ---
            groups = [list(range(g, min(g + 3, NMB))) for g in range(0, NMB, 3)]
            out1n_all = big.tile([128, NQC, QC], F32R)

            def emit_st(qsl, group):
                st = st_pool.tile([128, 3, QC], F32, tag="st")
                for i, mb in enumerate(group):
                    nc.tensor.matmul(
                        st[:, i, :],
                        k4[32 * i:32 * (i + 1), mb * 128:(mb + 1) * 128],
                        qt4[32 * i:32 * (i + 1), qsl],
                        start=True, stop=True,
                        tile_position=(32 * i, 0),
                    )
                return st

            for qc in range(NQC):
                qsl = slice(qc * QC, (qc + 1) * QC)
                out1 = acc_pool.tile([128, QC], F32, tag="out1")
                cs = acc_pool.tile([128, QC], F32, tag="cs")

                st_cur = emit_st(qsl, groups[0])
                for g, group in enumerate(groups):
                    ng = len(group)
                    e = e_pool.tile([128, 3, QC], FP16, tag="e")
                    nc.scalar.activation(
                        out=e[:, 0:ng, :].rearrange("p a b -> p (a b)"),
                        in_=st_cur[:, 0:ng, :].rearrange("p a b -> p (a b)"),
                        func=EXP,
                    )
                    if qc == 0:
                        for mb in group:
                            emit_vt(mb)
                        if qt4_chunks_done < NQC:
                            emit_conv_chunk(qt4, wq_s, tq_s, qt4_chunks_done)
                            qt4_chunks_done += 1
                    if g + 1 < len(groups):
                        if qc == 0:
                            need = (384 * (g + 2) - 1) // QC + 1
                            while k4_chunks_done < min(need, NQC):
                                emit_conv_chunk(k4, wk_s, tk_s, k4_chunks_done)
                                k4_chunks_done += 1
                        st_cur = emit_st(qsl, groups[g + 1])
                    for i, mb in enumerate(group):
                        nc.tensor.matmul(
                            out1, vt[:, mb, :], e[:, i, :],
                            start=(mb == 0), stop=(mb == NMB - 1),
                            skip_group_check=True,
                        )
                    for i, mb in enumerate(group):
                        j = mb % 4
                        nc.tensor.matmul(
                            cs[32 * j:32 * (j + 1), :],
                            ones[:, 0:32], e[:, i, :],
                            start=(mb < 4), stop=(mb >= NMB - 4),
                            tile_position=(0, 32 * j),
                            skip_group_check=True,
                        )

                # epilogue (SBUF-side; psum banks released by the two copies)
                cs_s = misc.tile([128, QC], F32, tag="cs_s")
                nc.vector.tensor_copy(cs_s, cs)
                out1_s = misc.tile([128, QC], F32, tag="out1_s")
                nc.vector.tensor_copy(out1_s, out1)
                slab = misc.tile([32, 3, QC], F32, tag="slab")
                for j in range(3):
                    nc.sync.dma_start(out=slab[:, j, :],
                                      in_=cs_s[32 * (j + 1):32 * (j + 2), :])
                t1 = misc.tile([32, QC], F32, tag="t1")
                nc.vector.tensor_add(t1, cs_s[0:32, :], slab[:, 0, :])
                t2 = misc.tile([32, QC], F32, tag="t2")
                nc.vector.tensor_add(t2, slab[:, 1, :], slab[:, 2, :])
                tot = misc.tile([32, QC], F32, tag="tot")
                nc.vector.tensor_add(tot, t1, t2)
                recipb = misc.tile([128, QC], F32, tag="recipb")
                nc.vector.reciprocal(recipb[0:32, :], tot)
                for j in range(1, 4):
                    nc.sync.dma_start(out=recipb[32 * j:32 * (j + 1), :],
                                      in_=recipb[0:32, :])
                nc.vector.tensor_mul(out1n_all[:, qc, :], out1_s, recipb)

            # --- projection tail: z[ob] = Wp_h[ob].T @ out1n (+bias) ---
            for qc in range(NQC):
                qsl = slice(qc * QC, (qc + 1) * QC)
                zp = st_pool.tile([128, 3, QC], F32, tag="st")
                for ob in range(2):
                    nc.tensor.matmul(zp[:, ob, :], wp_s[:, ob, :],
                                     out1n_all[:, qc, :],
                                     start=True, stop=True)
                zs = misc.tile([128, 2, QC], F32, tag="zs")
                for ob in range(2):
                    if qc % 2 == 0:
                        nc.vector.tensor_scalar_add(
                            zs[:, ob, :], zp[:, ob, :], bias_s[:, ob:ob + 1])
                    else:
                        nc.scalar.add(zs[:, ob, :], zp[:, ob, :],
                                      bias_s[:, ob:ob + 1])
                    nc.sync.dma_start(out=z[ob, :, qsl], in_=zs[:, ob, :])
    nc.compile()
    return nc


def _prepare_in_maps(x, wq, sq, tq, wk, sk, tk, wv, sv, tv, wp, sp, tp):
    X = np.ascontiguousarray(x.reshape(C, N), dtype=np.float32)
    x2 = np.ascontiguousarray(X.reshape(2, 128, N).transpose(1, 0, 2))
    bias_full = (sp * (wp @ tv) + tp) / H  # [256], per-core share

    in_maps = []
    for h in range(H):
        wq_h = (wq[32 * h:32 * (h + 1), :] * sq[32 * h:32 * (h + 1), None])
        wk_h = (wk[32 * h:32 * (h + 1), :] * sk[32 * h:32 * (h + 1), None])
        wqT_rep = np.tile(wq_h.T, (1, 4))        # [256, 128]
        wkT_rep = np.tile(wk_h.T, (1, 4))
        wq_r = np.ascontiguousarray(
            wqT_rep.reshape(2, 128, 128).transpose(1, 0, 2), dtype=np.float32)
        wk_r = np.ascontiguousarray(
            wkT_rep.reshape(2, 128, 128).transpose(1, 0, 2), dtype=np.float32)
        tq4 = np.ascontiguousarray(
            np.tile(tq[32 * h:32 * (h + 1)], 4).reshape(128, 1), dtype=np.float32)
        tk4 = np.ascontiguousarray(
            np.tile(tk[32 * h:32 * (h + 1)], 4).reshape(128, 1), dtype=np.float32)
        wv_h = wv[128 * h:128 * (h + 1), :]      # [dv, c]
        wv_r = np.ascontiguousarray(
            wv_h.T.reshape(2, 128, 128).transpose(1, 0, 2), dtype=np.float32)
        Wp_h = (sp[:, None] * wp[:, 128 * h:128 * (h + 1)]
                * sv[None, 128 * h:128 * (h + 1)])   # [256, 128]
        wp_r = np.ascontiguousarray(
            Wp_h.reshape(2, 128, 128).transpose(2, 0, 1), dtype=np.float32)
        bias2 = np.ascontiguousarray(
            bias_full.reshape(2, 128).T, dtype=np.float32)
        in_maps.append({
            "x2": x2, "wq_r": wq_r, "tq4": tq4, "wk_r": wk_r, "tk4": tk4,
            "wv_r": wv_r, "wp_r": wp_r, "bias2": bias2,
        })
    return in_maps


def kernel(**inputs):
    if "nc" not in _CACHED:
        _CACHED["nc"] = _build_program()
    nc = _CACHED["nc"]
    in_maps = _prepare_in_maps(**inputs)
    res = run_bass_kernel_spmd(nc, in_maps, core_ids=list(range(H)))
    y = np.zeros((2, 128, N), dtype=np.float64)
    for r in res.results:
        y += r["z"]
    return y.reshape(C, N).reshape(1, C, 64, 64).astype(np.float32)


# revision 7
# speedup vs baseline: 2.3330x; 1.0320x over previous
"""Trainium2 Bass kernel for nn_Attention (dense transformer attention block).

Reference computation (shapes hardcoded):
  x [1, 256, 64, 64]; 1x1 conv+BN to q/k/v; 8 heads; per-head
  softmax(q @ k) @ v over n=4096 positions (dqk=32, dv=128); 1x1 conv+BN proj.

Sharding: one head per NeuronCore (8 cores). Each core computes its head's
attention plus its partial contribution to the projection conv
(z_h = Wp_h @ out_h); the host sums the 8 partials (the only cross-head
coupling is the channel-concat feeding the proj conv, which is a sum of
per-head matmuls).

Folding tricks (host-side, exact):
 - BN scales sq/sk fold into wq/wk rows; biases tq/tk applied on-device
   per-partition.
 - v's BN affine (sv, tv) folds through the attention (softmax rows sum to 1):
   sv scales wp columns, tv contributes a constant bias via wp @ tv.
 - proj BN affine (sp, tp) folds into wp rows / the constant bias.

On-device layouts avoid all transposes:
 - QT = [32, n] and K = [32, n] conv outputs replicated 4x along partitions
   so S^T tiles can be computed with 4-way row-packed (tile_position) matmuls.
 - S^T blocks [128 m, 512 q] -> exp (ScalarE) -> E fp16.
 - V computed directly transposed ([m, dv] blocks) via lhsT=X.
 - PV accumulates over m in PSUM; softmax denominator via col-packed
   ones-matmul (colsum) of E; normalization folded into the proj stage.

Matmul dtypes: float32r (full-rate fp32, ~1e-4 component error) for
QK/V/proj; fp16 for E/V in the PV and colsum passes (~3e-4).
"""

import numpy as np

import concourse.bacc as bacc
import concourse.mybir as mybir
import concourse.tile as tile
from concourse.bass_utils import run_bass_kernel_spmd

F32 = mybir.dt.float32
F32R = mybir.dt.float32r
FP16 = mybir.dt.float16
EXP = mybir.ActivationFunctionType.Exp

N = 4096          # positions (64*64)
C = 256           # input channels
H = 8             # heads
DQK = 32
DV = 128
NMB = N // 128    # 32 m-blocks
NQC = 8           # q-chunks of 512
QC = 512

_CACHED = {}


def _build_program():
    nc = bacc.Bacc(trn_type="TRN2")

    x2 = nc.dram_tensor("x2", [128, 2, N], F32R, kind="ExternalInput")
    wq_r = nc.dram_tensor("wq_r", [128, 2, 128], F32R, kind="ExternalInput")
    tq4 = nc.dram_tensor("tq4", [128, 1], F32, kind="ExternalInput")
    wk_r = nc.dram_tensor("wk_r", [128, 2, 128], F32R, kind="ExternalInput")
    tk4 = nc.dram_tensor("tk4", [128, 1], F32, kind="ExternalInput")
    wv_r = nc.dram_tensor("wv_r", [128, 2, 128], F32R, kind="ExternalInput")
    wp_r = nc.dram_tensor("wp_r", [128, 2, 128], F32R, kind="ExternalInput")
    bias2 = nc.dram_tensor("bias2", [128, 2], F32, kind="ExternalInput")
    z = nc.dram_tensor("z", [2, 128, N], F32, kind="ExternalOutput")

    with tile.TileContext(nc) as tc:
        with (
            tc.tile_pool(name="wgt", bufs=1) as wgt,
            tc.tile_pool(name="big", bufs=1) as big,
            tc.tile_pool(name="st_p", bufs=2, space="PSUM") as st_pool,
            tc.tile_pool(name="acc_p", bufs=1, space="PSUM") as acc_pool,
            tc.tile_pool(name="e_p", bufs=3) as e_pool,
            tc.tile_pool(name="misc", bufs=2) as misc,
        ):
            wq_s = wgt.tile([128, 2, 128], F32R)
            nc.sync.dma_start(out=wq_s, in_=wq_r[:, :, :])
            wk_s = wgt.tile([128, 2, 128], F32R)
            nc.sync.dma_start(out=wk_s, in_=wk_r[:, :, :])
            wv_s = wgt.tile([128, 2, 128], F32R)
            nc.sync.dma_start(out=wv_s, in_=wv_r[:, :, :])
            wp_s = wgt.tile([128, 2, 128], F32R)
            nc.sync.dma_start(out=wp_s, in_=wp_r[:, :, :])
            tq_s = wgt.tile([128, 1], F32)
            nc.sync.dma_start(out=tq_s, in_=tq4[:, :])
            tk_s = wgt.tile([128, 1], F32)
            nc.sync.dma_start(out=tk_s, in_=tk4[:, :])
            bias_s = wgt.tile([128, 2], F32)
            nc.sync.dma_start(out=bias_s, in_=bias2[:, :])
            x2_s = wgt.tile([128, 2, N], F32R)
            for nq in range(4):
                nc.sync.dma_start(
                    out=x2_s[:, :, nq * 1024:(nq + 1) * 1024],
                    in_=x2[:, :, nq * 1024:(nq + 1) * 1024])

            ones_f = wgt.tile([128, 32], F32)
            nc.vector.memset(ones_f, 1.0)
            ones = wgt.tile([128, 32], FP16)
            nc.vector.tensor_copy(ones, ones_f)

            # --- q/k 1x1 convs (+ bias), replicated 4x on partitions.
            # Only the chunks needed for q-chunk 0 / trio 0 are emitted up
            # front; the rest stream into chunk 0's m-loop.
            qt4 = big.tile([128, N], F32R)
            k4 = big.tile([128, N], F32R)

            def emit_conv_chunk(dst, w_s, t_s, n8):
                sl = slice(n8 * QC, (n8 + 1) * QC)
                cp = st_pool.tile([128, QC], F32, tag="st")
                nc.tensor.matmul(cp, w_s[:, 0, :], x2_s[:, 0, sl],
                                 start=True, stop=False)
                nc.tensor.matmul(cp, w_s[:, 1, :], x2_s[:, 1, sl],
                                 start=False, stop=True)
                nc.vector.tensor_scalar_add(dst[:, sl], cp, t_s[:, 0:1])

            emit_conv_chunk(qt4, wq_s, tq_s, 0)
            emit_conv_chunk(k4, wk_s, tk_s, 0)
            k4_chunks_done = 1
            qt4_chunks_done = 1

            # --- V tiles [m, dv] fp16, computed inside chunk 0's m-loop ---
            vt = big.tile([128, NMB, DV], FP16)

            def emit_vt(mb):
                msl = slice(mb * 128, (mb + 1) * 128)
                vp = st_pool.tile([128, DV], F32, tag="st")
                nc.tensor.matmul(vp, x2_s[:, 0, msl], wv_s[:, 0, :],
                                 start=True, stop=False)
                nc.tensor.matmul(vp, x2_s[:, 1, msl], wv_s[:, 1, :],
                                 start=False, stop=True)
                nc.vector.tensor_copy(vt[:, mb, :], vp)

            # --- main attention loop over q-chunks (software-pipelined) ---
            groups = [list(range(g, min(g + 3, NMB))) for g in range(0, NMB, 3)]
            out1n_all = big.tile([128, NQC, QC], F32R)

            def emit_st(qsl, group):
                st = st_pool.tile([128, 3, QC], F32, tag="st")
                for i, mb in enumerate(group):
                    nc.tensor.matmul(
                        st[:, i, :],
                        k4[32 * i:32 * (i + 1), mb * 128:(mb + 1) * 128],
                        qt4[32 * i:32 * (i + 1), qsl],
                        start=True, stop=True,
                        tile_position=(32 * i, 0),
                    )
                return st

            for qc in range(NQC):
                qsl = slice(qc * QC, (qc + 1) * QC)
                out1 = acc_pool.tile([128, QC], F32, tag="out1")
                cs = acc_pool.tile([128, QC], F32, tag="cs")

                # depth-2 software pipeline: while exp(g) runs on ScalarE,
                # the PE computes ST(g+1) and PV/CS of g-1 (whose E is ready).
                ng_ = len(groups)
                sts = [None] * ng_
                es = [None] * ng_

                def emit_pvcs(g):
                    group = groups[g]
                    e = es[g]
                    for i, mb in enumerate(group):
                        nc.tensor.matmul(
                            out1, vt[:, mb, :], e[:, i, :],
                            start=(mb == 0), stop=(mb == NMB - 1),
                            skip_group_check=True,
                        )
                    for i, mb in enumerate(group):
                        j = mb % 4
                        nc.tensor.matmul(
                            cs[32 * j:32 * (j + 1), :],
                            ones[:, 0:32], e[:, i, :],
                            start=(mb < 4), stop=(mb >= NMB - 4),
                            tile_position=(0, 32 * j),
                            skip_group_check=True,
                        )

                sts[0] = emit_st(qsl, groups[0])
                for g, group in enumerate(groups):
                    ng = len(group)
                    e = e_pool.tile([128, 3, QC], FP16, tag="e")
                    es[g] = e
                    nc.scalar.activation(
                        out=e[:, 0:ng, :].rearrange("p a b -> p (a b)"),
                        in_=sts[g][:, 0:ng, :].rearrange("p a b -> p (a b)"),
                        func=EXP,
                    )
                    if qc == 0:
                        for mb in group:
                            emit_vt(mb)
                        if qt4_chunks_done < NQC:
                            emit_conv_chunk(qt4, wq_s, tq_s, qt4_chunks_done)
                            qt4_chunks_done += 1
                    if g + 1 < ng_:
                        if qc == 0:
                            need = (384 * (g + 2) - 1) // QC + 1
                            while k4_chunks_done < min(need, NQC):
                                emit_conv_chunk(k4, wk_s, tk_s, k4_chunks_done)
                                k4_chunks_done += 1
                        sts[g + 1] = emit_st(qsl, groups[g + 1])
                    if g >= 1:
                        emit_pvcs(g - 1)
                emit_pvcs(ng_ - 1)

                # epilogue (SBUF-side; psum banks released by the two copies)
                cs_s = misc.tile([128, QC], F32, tag="cs_s")
                nc.vector.tensor_copy(cs_s, cs)
                out1_s = misc.tile([128, QC], F32, tag="out1_s")
                nc.vector.tensor_copy(out1_s, out1)
                slab = misc.tile([32, 3, QC], F32, tag="slab")
                for j in range(3):
                    nc.sync.dma_start(out=slab[:, j, :],
                                      in_=cs_s[32 * (j + 1):32 * (j + 2), :])
                t1 = misc.tile([32, QC], F32, tag="t1")
                nc.vector.tensor_add(t1, cs_s[0:32, :], slab[:, 0, :])
                t2 = misc.tile([32, QC], F32, tag="t2")
                nc.vector.tensor_add(t2, slab[:, 1, :], slab[:, 2, :])
                tot = misc.tile([32, QC], F32, tag="tot")
                nc.vector.tensor_add(tot, t1, t2)
                recipb = misc.tile([128, QC], F32, tag="recipb")
                nc.vector.reciprocal(recipb[0:32, :], tot)
                for j in range(1, 4):
                    nc.sync.dma_start(out=recipb[32 * j:32 * (j + 1), :],
                                      in_=recipb[0:32, :])
                nc.vector.tensor_mul(out1n_all[:, qc, :], out1_s, recipb)

            # --- projection tail: z[ob] = Wp_h[ob].T @ out1n (+bias) ---
            for qc in range(NQC):
                qsl = slice(qc * QC, (qc + 1) * QC)
                zp = st_pool.tile([128, 3, QC], F32, tag="st")
                for ob in range(2):
                    nc.tensor.matmul(zp[:, ob, :], wp_s[:, ob, :],
                                     out1n_all[:, qc, :],
                                     start=True, stop=True)
                zs = misc.tile([128, 2, QC], F32, tag="zs")
                for ob in range(2):
                    if qc % 2 == 0:
                        nc.vector.tensor_scalar_add(
                            zs[:, ob, :], zp[:, ob, :], bias_s[:, ob:ob + 1])
                    else:
                        nc.scalar.add(zs[:, ob, :], zp[:, ob, :],
                                      bias_s[:, ob:ob + 1])
                    nc.sync.dma_start(out=z[ob, :, qsl], in_=zs[:, ob, :])
    nc.compile()
    return nc


def _prepare_in_maps(x, wq, sq, tq, wk, sk, tk, wv, sv, tv, wp, sp, tp):
    X = np.ascontiguousarray(x.reshape(C, N), dtype=np.float32)
    x2 = np.ascontiguousarray(X.reshape(2, 128, N).transpose(1, 0, 2))
    bias_full = (sp * (wp @ tv) + tp) / H  # [256], per-core share

    in_maps = []
    for h in range(H):
        wq_h = (wq[32 * h:32 * (h + 1), :] * sq[32 * h:32 * (h + 1), None])
        wk_h = (wk[32 * h:32 * (h + 1), :] * sk[32 * h:32 * (h + 1), None])
        wqT_rep = np.tile(wq_h.T, (1, 4))        # [256, 128]
        wkT_rep = np.tile(wk_h.T, (1, 4))
        wq_r = np.ascontiguousarray(
            wqT_rep.reshape(2, 128, 128).transpose(1, 0, 2), dtype=np.float32)
        wk_r = np.ascontiguousarray(
            wkT_rep.reshape(2, 128, 128).transpose(1, 0, 2), dtype=np.float32)
        tq4 = np.ascontiguousarray(
            np.tile(tq[32 * h:32 * (h + 1)], 4).reshape(128, 1), dtype=np.float32)
        tk4 = np.ascontiguousarray(
            np.tile(tk[32 * h:32 * (h + 1)], 4).reshape(128, 1), dtype=np.float32)
        wv_h = wv[128 * h:128 * (h + 1), :]      # [dv, c]
        wv_r = np.ascontiguousarray(
            wv_h.T.reshape(2, 128, 128).transpose(1, 0, 2), dtype=np.float32)
        Wp_h = (sp[:, None] * wp[:, 128 * h:128 * (h + 1)]
                * sv[None, 128 * h:128 * (h + 1)])   # [256, 128]
        wp_r = np.ascontiguousarray(
            Wp_h.reshape(2, 128, 128).transpose(2, 0, 1), dtype=np.float32)
        bias2 = np.ascontiguousarray(
            bias_full.reshape(2, 128).T, dtype=np.float32)
        in_maps.append({
            "x2": x2, "wq_r": wq_r, "tq4": tq4, "wk_r": wk_r, "tk4": tk4,
            "wv_r": wv_r, "wp_r": wp_r, "bias2": bias2,
        })
    return in_maps


def kernel(**inputs):
    if "nc" not in _CACHED:
        _CACHED["nc"] = _build_program()
    nc = _CACHED["nc"]
    in_maps = _prepare_in_maps(**inputs)
    res = run_bass_kernel_spmd(nc, in_maps, core_ids=list(range(H)))
    y = np.zeros((2, 128, N), dtype=np.float64)
    for r in res.results:
        y += r["z"]
    return y.reshape(C, N).reshape(1, C, 64, 64).astype(np.float32)


# revision 8
# speedup vs baseline: 2.6592x; 1.1398x over previous
"""Trainium2 Bass kernel for nn_Attention (dense transformer attention block).

Reference computation (shapes hardcoded):
  x [1, 256, 64, 64]; 1x1 conv+BN to q/k/v; 8 heads; per-head
  softmax(q @ k) @ v over n=4096 positions (dqk=32, dv=128); 1x1 conv+BN proj.

Sharding: one head per NeuronCore (8 cores). Each core computes its head's
attention plus its partial contribution to the projection conv
(z_h = Wp_h @ out_h); the host sums the 8 partials (the only cross-head
coupling is the channel-concat feeding the proj conv, which is a sum of
per-head matmuls).

Folding tricks (host-side, exact):
 - BN scales sq/sk fold into wq/wk rows; biases tq/tk applied on-device
   per-partition.
 - v's BN affine (sv, tv) folds through the attention (softmax rows sum to 1):
   sv scales wp columns, tv contributes a constant bias via wp @ tv.
 - proj BN affine (sp, tp) folds into wp rows / the constant bias.

On-device layouts avoid all transposes:
 - QT = [32, n] and K = [32, n] conv outputs replicated 4x along partitions
   so S^T tiles can be computed with 4-way row-packed (tile_position) matmuls.
 - S^T blocks [128 m, 512 q] -> exp (ScalarE) -> E fp16.
 - V computed directly transposed ([m, dv] blocks) via lhsT=X.
 - PV accumulates over m in PSUM; softmax denominator via col-packed
   ones-matmul (colsum) of E; normalization folded into the proj stage.

Matmul dtypes: float32r (full-rate fp32, ~1e-4 component error) for
QK/V/proj; fp16 for E/V in the PV and colsum passes (~3e-4).
"""

import numpy as np

import concourse.bacc as bacc
import concourse.mybir as mybir
import concourse.tile as tile
from concourse.bass_utils import run_bass_kernel_spmd

F32 = mybir.dt.float32
F32R = mybir.dt.float32r
FP16 = mybir.dt.float16
EXP = mybir.ActivationFunctionType.Exp

N = 4096          # positions (64*64)
C = 256           # input channels
H = 8             # heads
DQK = 32
DV = 128
NMB = N // 128    # 32 m-blocks
NQC = 8           # q-chunks of 512
QC = 512

_CACHED = {}


def _build_program():
    nc = bacc.Bacc(trn_type="TRN2")

    x2 = nc.dram_tensor("x2", [128, 2, N], F32R, kind="ExternalInput")
    x2h = nc.dram_tensor("x2h", [128, 2, N], FP16, kind="ExternalInput")
    wq_r = nc.dram_tensor("wq_r", [128, 2, 128], F32R, kind="ExternalInput")
    tq4 = nc.dram_tensor("tq4", [128, 1], F32, kind="ExternalInput")
    wk_r = nc.dram_tensor("wk_r", [128, 2, 128], F32R, kind="ExternalInput")
    tk4 = nc.dram_tensor("tk4", [128, 1], F32, kind="ExternalInput")
    wv_r = nc.dram_tensor("wv_r", [128, 2, 128], FP16, kind="ExternalInput")
    wp_r = nc.dram_tensor("wp_r", [128, 2, 128], F32R, kind="ExternalInput")
    bias2 = nc.dram_tensor("bias2", [128, 2], F32, kind="ExternalInput")
    z = nc.dram_tensor("z", [2, 128, N], F32, kind="ExternalOutput")

    with tile.TileContext(nc) as tc:
        with (
            tc.tile_pool(name="wgt", bufs=1) as wgt,
            tc.tile_pool(name="big", bufs=1) as big,
            tc.tile_pool(name="st_p", bufs=2, space="PSUM") as st_pool,
            tc.tile_pool(name="acc_p", bufs=1, space="PSUM") as acc_pool,
            tc.tile_pool(name="e_p", bufs=3) as e_pool,
            tc.tile_pool(name="misc", bufs=2) as misc,
        ):
            wq_s = wgt.tile([128, 2, 128], F32R)
            nc.sync.dma_start(out=wq_s, in_=wq_r[:, :, :])
            wk_s = wgt.tile([128, 2, 128], F32R)
            nc.sync.dma_start(out=wk_s, in_=wk_r[:, :, :])
            wv_s = wgt.tile([128, 2, 128], FP16)
            nc.sync.dma_start(out=wv_s, in_=wv_r[:, :, :])
            wp_s = wgt.tile([128, 2, 128], F32R)
            nc.sync.dma_start(out=wp_s, in_=wp_r[:, :, :])
            tq_s = wgt.tile([128, 1], F32)
            nc.sync.dma_start(out=tq_s, in_=tq4[:, :])
            tk_s = wgt.tile([128, 1], F32)
            nc.sync.dma_start(out=tk_s, in_=tk4[:, :])
            bias_s = wgt.tile([128, 2], F32)
            nc.sync.dma_start(out=bias_s, in_=bias2[:, :])
            x2_s = wgt.tile([128, 2, N], F32R)
            x2h_s = wgt.tile([128, 2, N], FP16)
            for nq in range(4):
                nc.sync.dma_start(
                    out=x2_s[:, :, nq * 1024:(nq + 1) * 1024],
                    in_=x2[:, :, nq * 1024:(nq + 1) * 1024])
                nc.sync.dma_start(
                    out=x2h_s[:, :, nq * 1024:(nq + 1) * 1024],
                    in_=x2h[:, :, nq * 1024:(nq + 1) * 1024])

            ones_f = wgt.tile([128, 32], F32)
            nc.vector.memset(ones_f, 1.0)
            ones = wgt.tile([128, 32], FP16)
            nc.vector.tensor_copy(ones, ones_f)

            # --- q/k 1x1 convs (+ bias), replicated 4x on partitions.
            # Only the chunks needed for q-chunk 0 / trio 0 are emitted up
            # front; the rest stream into chunk 0's m-loop.
            qt4 = big.tile([128, N], FP16)
            k4 = big.tile([128, N], FP16)

            def emit_conv_chunk(dst, w_s, t_s, n8):
                sl = slice(n8 * QC, (n8 + 1) * QC)
                cp = st_pool.tile([128, QC], F32, tag="st")
                nc.tensor.matmul(cp, w_s[:, 0, :], x2_s[:, 0, sl],
                                 start=True, stop=False)
                nc.tensor.matmul(cp, w_s[:, 1, :], x2_s[:, 1, sl],
                                 start=False, stop=True)
                nc.vector.tensor_scalar_add(dst[:, sl], cp, t_s[:, 0:1])

            emit_conv_chunk(qt4, wq_s, tq_s, 0)
            emit_conv_chunk(k4, wk_s, tk_s, 0)
            k4_chunks_done = 1
            qt4_chunks_done = 1

            # --- V tiles [m, dv] fp16, computed inside chunk 0's m-loop ---
            vt = big.tile([128, NMB, DV], FP16)

            def emit_vt(mb):
                msl = slice(mb * 128, (mb + 1) * 128)
                vp = st_pool.tile([128, DV], F32, tag="st")
                nc.tensor.matmul(vp, x2h_s[:, 0, msl], wv_s[:, 0, :],
                                 start=True, stop=False)
                nc.tensor.matmul(vp, x2h_s[:, 1, msl], wv_s[:, 1, :],
                                 start=False, stop=True)
                nc.vector.tensor_copy(vt[:, mb, :], vp)

            # --- main attention loop over q-chunks (software-pipelined) ---
            groups = [list(range(g, min(g + 3, NMB))) for g in range(0, NMB, 3)]
            out1n_all = big.tile([128, NQC, QC], F32R)

            def emit_st(qsl, group):
                st = st_pool.tile([128, 3, QC], F32, tag="st")
                for i, mb in enumerate(group):
                    nc.tensor.matmul(
                        st[:, i, :],
                        k4[32 * i:32 * (i + 1), mb * 128:(mb + 1) * 128],
                        qt4[32 * i:32 * (i + 1), qsl],
                        start=True, stop=True,
                        tile_position=(32 * i, 0),
                    )
                return st

            for qc in range(NQC):
                qsl = slice(qc * QC, (qc + 1) * QC)
                out1 = acc_pool.tile([128, QC], F32, tag="out1")
                cs = acc_pool.tile([128, QC], F32, tag="cs")

                # depth-2 software pipeline: while exp(g) runs on ScalarE,
                # the PE computes ST(g+1) and PV/CS of g-1 (whose E is ready).
                ng_ = len(groups)
                sts = [None] * ng_
                es = [None] * ng_

                def emit_pvcs(g):
                    group = groups[g]
                    e = es[g]
                    for i, mb in enumerate(group):
                        nc.tensor.matmul(
                            out1, vt[:, mb, :], e[:, i, :],
                            start=(mb == 0), stop=(mb == NMB - 1),
                            skip_group_check=True,
                        )
                    for i, mb in enumerate(group):
                        j = mb % 4
                        nc.tensor.matmul(
                            cs[32 * j:32 * (j + 1), :],
                            ones[:, 0:32], e[:, i, :],
                            start=(mb < 4), stop=(mb >= NMB - 4),
                            tile_position=(0, 32 * j),
                            skip_group_check=True,
                        )

                sts[0] = emit_st(qsl, groups[0])
                for g, group in enumerate(groups):
                    ng = len(group)
                    e = e_pool.tile([128, 3, QC], FP16, tag="e")
                    es[g] = e
                    nc.scalar.activation(
                        out=e[:, 0:ng, :].rearrange("p a b -> p (a b)"),
                        in_=sts[g][:, 0:ng, :].rearrange("p a b -> p (a b)"),
                        func=EXP,
                    )
                    if qc == 0:
                        for mb in group:
                            emit_vt(mb)
                        if qt4_chunks_done < NQC:
                            emit_conv_chunk(qt4, wq_s, tq_s, qt4_chunks_done)
                            qt4_chunks_done += 1
                    if g + 1 < ng_:
                        if qc == 0:
                            need = (384 * (g + 2) - 1) // QC + 1
                            while k4_chunks_done < min(need, NQC):
                                emit_conv_chunk(k4, wk_s, tk_s, k4_chunks_done)
                                k4_chunks_done += 1
                        sts[g + 1] = emit_st(qsl, groups[g + 1])
                    if g >= 1:
                        emit_pvcs(g - 1)
                emit_pvcs(ng_ - 1)

                # epilogue (SBUF-side; psum banks released by the two copies)
                cs_s = misc.tile([128, QC], F32, tag="cs_s")
                nc.vector.tensor_copy(cs_s, cs)
                out1_s = misc.tile([128, QC], F32, tag="out1_s")
                nc.vector.tensor_copy(out1_s, out1)
                slab = misc.tile([32, 3, QC], F32, tag="slab")
                for j in range(3):
                    nc.sync.dma_start(out=slab[:, j, :],
                                      in_=cs_s[32 * (j + 1):32 * (j + 2), :])
                t1 = misc.tile([32, QC], F32, tag="t1")
                nc.vector.tensor_add(t1, cs_s[0:32, :], slab[:, 0, :])
                t2 = misc.tile([32, QC], F32, tag="t2")
                nc.vector.tensor_add(t2, slab[:, 1, :], slab[:, 2, :])
                tot = misc.tile([32, QC], F32, tag="tot")
                nc.vector.tensor_add(tot, t1, t2)
                recipb = misc.tile([128, QC], F32, tag="recipb")
                nc.vector.reciprocal(recipb[0:32, :], tot)
                for j in range(1, 4):
                    nc.sync.dma_start(out=recipb[32 * j:32 * (j + 1), :],
                                      in_=recipb[0:32, :])
                nc.vector.tensor_mul(out1n_all[:, qc, :], out1_s, recipb)

            # --- projection tail: z[ob] = Wp_h[ob].T @ out1n (+bias) ---
            for qc in range(NQC):
                qsl = slice(qc * QC, (qc + 1) * QC)
                zp = st_pool.tile([128, 3, QC], F32, tag="st")
                for ob in range(2):
                    nc.tensor.matmul(zp[:, ob, :], wp_s[:, ob, :],
                                     out1n_all[:, qc, :],
                                     start=True, stop=True)
                zs = misc.tile([128, 2, QC], F32, tag="zs")
                for ob in range(2):
                    if qc % 2 == 0:
                        nc.vector.tensor_scalar_add(
                            zs[:, ob, :], zp[:, ob, :], bias_s[:, ob:ob + 1])
                    else:
                        nc.scalar.add(zs[:, ob, :], zp[:, ob, :],
                                      bias_s[:, ob:ob + 1])
                    nc.sync.dma_start(out=z[ob, :, qsl], in_=zs[:, ob, :])
    nc.compile()
    return nc


def _prepare_in_maps(x, wq, sq, tq, wk, sk, tk, wv, sv, tv, wp, sp, tp):
    X = np.ascontiguousarray(x.reshape(C, N), dtype=np.float32)
    x2 = np.ascontiguousarray(X.reshape(2, 128, N).transpose(1, 0, 2))
    x2h = x2.astype(np.float16)
    bias_full = (sp * (wp @ tv) + tp) / H  # [256], per-core share

    in_maps = []
    for h in range(H):
        wq_h = (wq[32 * h:32 * (h + 1), :] * sq[32 * h:32 * (h + 1), None])
        wk_h = (wk[32 * h:32 * (h + 1), :] * sk[32 * h:32 * (h + 1), None])
        wqT_rep = np.tile(wq_h.T, (1, 4))        # [256, 128]
        wkT_rep = np.tile(wk_h.T, (1, 4))
        wq_r = np.ascontiguousarray(
            wqT_rep.reshape(2, 128, 128).transpose(1, 0, 2), dtype=np.float32)
        wk_r = np.ascontiguousarray(
            wkT_rep.reshape(2, 128, 128).transpose(1, 0, 2), dtype=np.float32)
        tq4 = np.ascontiguousarray(
            np.tile(tq[32 * h:32 * (h + 1)], 4).reshape(128, 1), dtype=np.float32)
        tk4 = np.ascontiguousarray(
            np.tile(tk[32 * h:32 * (h + 1)], 4).reshape(128, 1), dtype=np.float32)
        wv_h = wv[128 * h:128 * (h + 1), :]      # [dv, c]
        wv_r = np.ascontiguousarray(
            wv_h.T.reshape(2, 128, 128).transpose(1, 0, 2), dtype=np.float32)
        Wp_h = (sp[:, None] * wp[:, 128 * h:128 * (h + 1)]
                * sv[None, 128 * h:128 * (h + 1)])   # [256, 128]
        wp_r = np.ascontiguousarray(
            Wp_h.reshape(2, 128, 128).transpose(2, 0, 1), dtype=np.float32)
        bias2 = np.ascontiguousarray(
            bias_full.reshape(2, 128).T, dtype=np.float32)
        in_maps.append({
            "x2": x2, "x2h": x2h, "wq_r": wq_r, "tq4": tq4, "wk_r": wk_r,
            "tk4": tk4, "wv_r": wv_r.astype(np.float16), "wp_r": wp_r,
            "bias2": bias2,
        })
    return in_maps


def kernel(**inputs):
    if "nc" not in _CACHED:
        _CACHED["nc"] = _build_program()
    nc = _CACHED["nc"]
    in_maps = _prepare_in_maps(**inputs)
    res = run_bass_kernel_spmd(nc, in_maps, core_ids=list(range(H)))
    y = np.zeros((2, 128, N), dtype=np.float64)
    for r in res.results:
        y += r["z"]
    return y.reshape(C, N).reshape(1, C, 64, 64).astype(np.float32)
